# revision 7
# baseline (speedup 1.0000x reference)
"""Causal self-attention with RoPE on 8 trn2 NeuronCores.

Sharding: core = (batch, head-half). Each of the 8 cores handles one batch
(b = core//2) and 6 of the 12 heads (hh = core%2). Each core computes a
partial output projection (its heads' contribution to y @ Wproj); the host
sums the two partials per batch.

v2 design (cost-model-driven):
  - qkv projections run as fp8e4 DoubleRow matmuls, 3-term hi-lo
    decomposition (x_hi*W_hi + x_lo*W_hi + x_hi*W_lo). W is pre-scaled by 64
    host-side so its hi/lo pair avoids the e4m3 subnormal floor; the 64x is
    compensated downstream (RoPE tables x0.25 -> q,k at 16x; v kept at 64x
    with a 64-valued ones column so softmax denominators cancel the scale).
  - RoPE via stream_shuffle (host-permuted even/odd partner lanes), output
    quantized to fp8: q as a single fp8 plane, k as an (hi, lo) fp8 pair.
  - scores: one DoubleRow matmul per (head, 128-k-block): stationary
    k = (hi, lo) planes, moving q8 broadcast with a stride-0 plane dim ->
    k-exact * q8 at 0.5 cycles/row.
  - exp on ACT in 2-k-block batches over 1024-wide q chunks (fused
    scale=1/2048, bias=-ln 16); causal mask applied as 0/1 multiplies on the
    diagonal blocks only.
  - EV in fp16 ([v64 | 64] ones-column trick gives the denominator row).
  - output projection in fp16 from yT (= y, scale-free after the reciprocal
    multiply) with natural-scale fp16 Wproj.
"""
import contextlib
import math

import numpy as np

import concourse.bacc as bacc
import concourse.mybir as mybir
import concourse.tile as tile
from concourse import bass_utils

F32 = mybir.dt.float32
F16 = mybir.dt.float16
F8 = mybir.dt.float8e4
DR = mybir.MatmulPerfMode.DoubleRow

B, S, C, H, D = 4, 2048, 768, 12, 64
HPC = H // 2          # heads per core = 6
HP = HPC // 2         # head pairs per core = 3
NSC = S // 512        # 512-wide s chunks for projections = 4
NST = S // 128        # 128-row s tiles = 16
NQC = S // 1024       # 1024-wide attention q chunks = 2
ROPE_BASE = 10000.0
WSCALE = 64.0         # host-side Wqkv/Wv scale (fp8 subnormal dodge)
TSCALE = 16.0 / WSCALE  # RoPE table scale -> q,k at 16x
EXP_SCALE = (D ** -0.5) / 256.0
EXP_BIAS = -math.log(16.0)
VONES = 64.0          # ones-column value (= v scale) so denom cancels

EVEN_ODD_MASK = [x for j in range(16) for x in (2 * j + 1, 2 * j)]


def build_program():
    nc = bacc.Bacc("TRN2", target_bir_lowering=False, debug=False)
    xh_d = nc.dram_tensor("xh", [128, 6, S], F8, kind="ExternalInput").ap()
    xl_d = nc.dram_tensor("xl", [128, 6, S], F8, kind="ExternalInput").ap()
    wqkh_d = nc.dram_tensor("wqkh", [128, 6, 768], F8, kind="ExternalInput").ap()
    wqkl_d = nc.dram_tensor("wqkl", [128, 6, 768], F8, kind="ExternalInput").ap()
    wvh_d = nc.dram_tensor("wvh", [128, 6, 384], F8, kind="ExternalInput").ap()
    wvl_d = nc.dram_tensor("wvl", [128, 6, 384], F8, kind="ExternalInput").ap()
    wp_d = nc.dram_tensor("wp", [384, C], F16, kind="ExternalInput").ap()
    cos_d = nc.dram_tensor("cosT", [128, S], F16, kind="ExternalInput").ap()
    sin_d = nc.dram_tensor("sinA", [128, S], F16, kind="ExternalInput").ap()
    mask_d = nc.dram_tensor("mask01", [128, 128], F16, kind="ExternalInput").ap()
    maskx_d = nc.dram_tensor("mask01x", [128, 256], F16, kind="ExternalInput").ap()
    out_d = nc.dram_tensor("out", [S, C], F32, kind="ExternalOutput").ap()

    with tile.TileContext(nc) as tc, contextlib.ExitStack() as top:
        sb = top.enter_context(tc.tile_pool(name="sb", bufs=1))
        ps = top.enter_context(tc.tile_pool(name="ps", bufs=1, space="PSUM"))

        # persistent SBUF tiles
        xh = [sb.tile([128, 2, S], F8, name=f"xh{t}", tag=f"xh{t}") for t in range(3)]
        xl = [sb.tile([128, 2, S], F8, name=f"xl{t}", tag=f"xl{t}") for t in range(3)]
        wqkh = [sb.tile([128, 2, 768], F8, name=f"wqkh{t}", tag="wqkh", bufs=3) for t in range(3)]
        wqkl = [sb.tile([128, 2, 768], F8, name=f"wqkl{t}", tag="wqkl", bufs=3) for t in range(3)]
        wvh = [sb.tile([128, 2, 384], F8, name=f"wvh{t}", tag="wvh", bufs=3) for t in range(3)]
        wvl = [sb.tile([128, 2, 384], F8, name=f"wvl{t}", tag="wvl", bufs=3) for t in range(3)]
        wp = [sb.tile([128, C], F16, name=f"wp{t}", tag="wp", bufs=3) for t in range(3)]
        cosT = sb.tile([128, S], F16, name="cosT", tag="cosT")
        sinA = sb.tile([128, S], F16, name="sinA", tag="sinA")
        mask01 = sb.tile([128, 128], F16, name="mask01", tag="mask01")
        mask01x = sb.tile([128, 256], F16, name="mask01x", tag="mask01x")
        ebias = sb.tile([128, 1], F32, name="ebias", tag="ebias")
        nc.gpsimd.memset(ebias[:], EXP_BIAS)
        q8 = [sb.tile([128, S], F8, name=f"q8{m}", tag=f"q8{m}") for m in range(3)]
        k8 = [sb.tile([128, 2, S], F8, name=f"k8{m}", tag=f"k8{m}") for m in range(3)]
        vones = [sb.tile([128, HPC, 65], F16, name=f"vones{i}", tag=f"vones{i}")
                 for i in range(NST)]
        yTn = [sb.tile([128, S], F16, name=f"yTn{t}", tag=f"yTn{t}") for t in range(3)]

        # input loads: weights + first x chunk first
        nc.sync.dma_start(mask01[:], mask_d[:])
        nc.sync.dma_start(mask01x[:], maskx_d[:])
        for t in range(3):
            nc.sync.dma_start(wqkh[t][:], wqkh_d[:, 2 * t:2 * t + 2, :])
            nc.sync.dma_start(wqkl[t][:], wqkl_d[:, 2 * t:2 * t + 2, :])
            nc.sync.dma_start(wvh[t][:], wvh_d[:, 2 * t:2 * t + 2, :])
            nc.sync.dma_start(wvl[t][:], wvl_d[:, 2 * t:2 * t + 2, :])
        for sc in range(NSC):
            sl = slice(512 * sc, 512 * (sc + 1))
            for t in range(3):
                nc.sync.dma_start(xh[t][:, :, sl], xh_d[:, 2 * t:2 * t + 2, sl])
                nc.sync.dma_start(xl[t][:, :, sl], xl_d[:, 2 * t:2 * t + 2, sl])
            nc.sync.dma_start(cosT[:, sl], cos_d[:, sl])
            nc.sync.dma_start(sinA[:, sl], sin_d[:, sl])
        for t in range(3):
            nc.sync.dma_start(wp[t][:], wp_d[128 * t:128 * (t + 1), :])

        def qk_mtile(m, sc):
            """(x @ Wq/Wk)^T m-tile for s-chunk sc, RoPE'd + fp8-quantized."""
            sl = slice(512 * sc, 512 * (sc + 1))
            msl = slice(128 * m, 128 * (m + 1))
            pq_t = ps.tile([128, 512], F32, name="pq", tag="pq", bufs=2)
            pq = pq_t[:, 0:512]
            n = 0
            for t in range(3):
                for lhs, rhs in ((wqkh[t], xh[t]), (wqkh[t], xl[t]), (wqkl[t], xh[t])):
                    nc.tensor.matmul(
                        pq, lhs[:, :, msl], rhs[:, :, sl],
                        start=(n == 0), stop=(n == 8), perf_mode=DR,
                    )
                    n += 1
            shuf = sb.tile([128, 512], F32, name="shuf", tag="shuf", bufs=3)
            nc.vector.stream_shuffle(shuf[:], pq, EVEN_ODD_MASK)
            qc = sb.tile([128, 512], F32, name="qc", tag="qc", bufs=3)
            nc.vector.tensor_mul(qc[:], pq, cosT[:, sl])
            nc.gpsimd.tensor_mul(shuf[:], shuf[:], sinA[:, sl])
            if m < 3:
                nc.vector.tensor_add(q8[m][:, sl], qc[:], shuf[:])
            else:
                kf = sb.tile([128, 512], F32, name="kf", tag="kf", bufs=2)
                nc.vector.tensor_add(kf[:], qc[:], shuf[:])
                nc.gpsimd.tensor_copy(k8[m - 3][:, 0, sl], kf[:])
                nc.gpsimd.tensor_sub(k8[m - 3][:, 1, sl], kf[:], k8[m - 3][:, 0, sl])

        def v_tile(st):
            """v s-tile (fp16 at 64x scale, with 64-valued ones column)."""
            ssl = slice(128 * st, 128 * (st + 1))
            pq_t = ps.tile([128, 512], F32, name="pq", tag="pq", bufs=2)
            vps = pq_t[:, 0:384]
            n = 0
            for t in range(3):
                for lhs, rhs in ((xh[t], wvh[t]), (xl[t], wvh[t]), (xh[t], wvl[t])):
                    nc.tensor.matmul(
                        vps, lhs[:, :, ssl], rhs[:],
                        start=(n == 0), stop=(n == 8), perf_mode=DR,
                    )
                    n += 1
            nc.vector.tensor_copy(
                vones[st][:, :, 0:64], vps.rearrange("p (h w) -> p h w", w=64)
            )
            nc.gpsimd.memset(vones[st][:, :, 64:65], VONES)

        def attn(h, c):
            """Causal attention for head h, q-chunk c (1024 wide)."""
            hp, hh = h // 2, h % 2
            prow = slice(64 * hh, 64 * hh + 64)
            ngrp = 4 + 4 * c
            yps_t = ps.tile([128, 1024], F32, name="yps", tag="yps", bufs=1)
            yps = yps_t[:, 0:1024]
            with tc.high_priority(offset=150):
                for g in range(ngrp):
                    off = max(0, 256 * g - 1024 * c)
                    diag = 256 * (g + 1) > 1024 * c  # group touches the diagonal
                    # matmul outputs must stay within one 512-f32 psum bank
                    segs = [(off, 512), (512, 1024)] if off < 512 else [(off, 1024)]
                    sT = ps.tile([128, 2, 1024], F32, name="sT", tag="sT", bufs=1)
                    for j in range(2):
                        kb = 2 * g + j
                        ksl = slice(128 * kb, 128 * (kb + 1))
                        for a, b in segs:
                            qmv = q8[hp][prow, 1024 * c + a:1024 * c + b]
                            nc.tensor.matmul(
                                sT[:, j, a:b], k8[hp][prow, :, ksl],
                                qmv.unsqueeze(1).broadcast_to([64, 2, b - a]),
                                start=True, stop=True, perf_mode=DR,
                                tile_position=(64 * hh, 0),
                            )
                    eT = sb.tile([128, 2, 1024], F16, name="eT", tag="eT", bufs=4)
                    nc.scalar.activation(
                        eT[:, :, off:1024], sT[:, :, off:1024],
                        mybir.ActivationFunctionType.Exp,
                        scale=EXP_SCALE, bias=ebias[:],
                    )
                    if diag:
                        nc.vector.tensor_mul(
                            eT[:, 0, off:off + 128], eT[:, 0, off:off + 128], mask01[:]
                        )
                        nc.gpsimd.tensor_mul(
                            eT[:, 1, off:off + 256], eT[:, 1, off:off + 256], mask01x[:]
                        )
                    off_next = max(0, 256 * (g + 1) - 1024 * c)
                    for j in range(2):
                        kb = 2 * g + j
                        for a, b in segs:
                            # stop must land on the LAST write of each psum
                            # bank (group tracking is per bank)
                            if b == 512:
                                stop = j == 1 and (off_next >= 512 or g == ngrp - 1)
                            else:
                                stop = j == 1 and g == ngrp - 1
                            nc.tensor.matmul(
                                yps[0:65, a:b], vones[kb][:, h, :],
                                eT[:, j, a:b],
                                start=(g == 0 and j == 0), stop=stop,
                            )
            # evacuate: yT = y / den (scale cancels), fp16
            recip = sb.tile([1, 1024], F32, name="recip", tag="recip", bufs=2)
            nc.vector.reciprocal(recip[:], yps[64:65, 0:1024])
            bc = sb.tile([64, 1024], F32, name="bc", tag="bc", bufs=2)
            nc.gpsimd.partition_broadcast(bc[:], recip[:], channels=64)
            nc.vector.tensor_mul(
                yTn[hp][prow, 1024 * c:1024 * (c + 1)], yps[0:64, 0:1024], bc[:]
            )

        def oproj(st):
            osb = sb.tile([128, C], F32, name="osb", tag="osb", bufs=2)
            for half in range(2):
                pq_t = ps.tile([128, 512], F32, name="pq", tag="pq", bufs=2)
                ops_ = pq_t[:, 0:384]
                for t in range(3):
                    nc.tensor.matmul(
                        ops_, yTn[t][:, 128 * st:128 * (st + 1)],
                        wp[t][:, 384 * half:384 * (half + 1)],
                        start=(t == 0), stop=(t == 2),
                    )
                nc.vector.tensor_copy(osb[:, 384 * half:384 * (half + 1)], ops_)
            nc.sync.dma_start(out_d[128 * st:128 * (st + 1), :], osb[:])

        # emission: attention as early as deps allow (program order defines
        # dataflow semantics), projections interleaved as PE filler
        qk_mtile(0, 0); qk_mtile(3, 0); qk_mtile(0, 1); qk_mtile(3, 1)
        for st in range(0, 8):
            v_tile(st)
        attn(0, 0)
        qk_mtile(1, 0); qk_mtile(4, 0); qk_mtile(1, 1); qk_mtile(4, 1)
        attn(1, 0)
        qk_mtile(2, 0); qk_mtile(5, 0); qk_mtile(2, 1); qk_mtile(5, 1)
        attn(2, 0)
        qk_mtile(0, 2); qk_mtile(3, 2)
        attn(3, 0)
        qk_mtile(0, 3); qk_mtile(3, 3)
        attn(4, 0)
        for st in range(8, 16):
            v_tile(st)
        attn(5, 0)
        attn(0, 1)
        qk_mtile(1, 2); qk_mtile(4, 2); qk_mtile(1, 3); qk_mtile(4, 3)
        attn(1, 1)
        qk_mtile(2, 2); qk_mtile(5, 2); qk_mtile(2, 3); qk_mtile(5, 3)
        attn(2, 1)
        oproj(0); oproj(1)
        attn(3, 1)
        oproj(2); oproj(3)
        attn(4, 1)
        oproj(4); oproj(5); oproj(6)
        attn(5, 1)
        oproj(7)
        for st in range(8, NST):
            oproj(st)

    nc.compile()
    return nc


def _rope_tables():
    """cosT/sinA fp16 tables (x TSCALE), even/odd-interleaved d order, tiled
    to 128 partitions (two 64-row head blocks)."""
    j = np.arange(32, dtype=np.float64)
    theta = ROPE_BASE ** (-2.0 * j / D)
    pos = np.arange(S, dtype=np.float64)
    freqs = np.outer(theta, pos)  # (32, S)
    cos = np.cos(freqs) * TSCALE
    sin = np.sin(freqs) * TSCALE
    cosT = np.empty((64, S), np.float32)
    sinA = np.empty((64, S), np.float32)
    cosT[0::2] = cos
    cosT[1::2] = cos
    sinA[0::2] = -sin
    sinA[1::2] = sin
    return np.tile(cosT, (2, 1)).copy(), np.tile(sinA, (2, 1)).copy()


def _head_perm():
    """Even/odd interleave of RoPE partner dims, per head (384 cols)."""
    perm = np.empty(384, np.int64)
    for h in range(HPC):
        for j in range(32):
            perm[64 * h + 2 * j] = 64 * h + j
            perm[64 * h + 2 * j + 1] = 64 * h + j + 32
    return perm


def _hilo(a):
    """Same-scale e4m3 hi/lo pair of a float32 array."""
    from ml_dtypes import float8_e4m3fn as e4m3
    hi = a.astype(e4m3)
    lo = (a - hi.astype(np.float32)).astype(e4m3)
    return hi, lo


def _dr_pack(w):
    """[768 channels, M] -> [128, 6, M] with (p, 2t+kt) = channel 256t+128kt+p."""
    M = w.shape[1]
    return np.ascontiguousarray(
        w.reshape(3, 2, 128, M).transpose(2, 0, 1, 3).reshape(128, 6, M)
    )


def make_in_maps(x, Wqkv, Wproj):
    x = np.asarray(x, np.float32)
    Wqkv = np.asarray(Wqkv, np.float32)
    Wproj = np.asarray(Wproj, np.float32)
    wq, wk, wv = Wqkv[:, 0:C], Wqkv[:, C:2 * C], Wqkv[:, 2 * C:3 * C]
    cosT, sinA = _rope_tables()
    perm = _head_perm()
    mask01 = (np.arange(128)[None, :] >= np.arange(128)[:, None]).astype(np.float16)
    mask01x = np.concatenate([np.zeros((128, 128), np.float16), mask01], axis=1)
    in_maps = []
    for core in range(8):
        b, hh = core // 2, core % 2
        cols = slice(384 * hh, 384 * (hh + 1))
        wqk_c = np.concatenate(
            [wq[:, cols][:, perm], wk[:, cols][:, perm]], axis=1
        ) * WSCALE  # [768, 768]
        wqk_hi, wqk_lo = _hilo(wqk_c)
        wv_c = wv[:, cols] * WSCALE
        wv_hi, wv_lo = _hilo(wv_c)
        xT = np.ascontiguousarray(x[b].T)  # [768, 2048]
        x_hi, x_lo = _hilo(xT)
        in_maps.append(
            {
                "xh": _dr_pack(x_hi),
                "xl": _dr_pack(x_lo),
                "wqkh": _dr_pack(wqk_hi),
                "wqkl": _dr_pack(wqk_lo),
                "wvh": _dr_pack(wv_hi),
                "wvl": _dr_pack(wv_lo),
                "wp": np.ascontiguousarray(
                    Wproj[384 * hh:384 * (hh + 1), :].astype(np.float16)
                ),
                "cosT": cosT.astype(np.float16),
                "sinA": sinA.astype(np.float16),
                "mask01": mask01,
                "mask01x": mask01x,
            }
        )
    return in_maps


_NC_CACHE = None


def _get_program():
    global _NC_CACHE
    if _NC_CACHE is None:
        _NC_CACHE = build_program()
    return _NC_CACHE


def kernel(x, Wqkv, Wproj):
    nc = _get_program()
    in_maps = make_in_maps(x, Wqkv, Wproj)
    res = bass_utils.run_bass_kernel_spmd(nc, in_maps, core_ids=list(range(8)))
    out = np.empty((B, S, C), np.float32)
    for b in range(B):
        out[b] = res.results[2 * b]["out"] + res.results[2 * b + 1]["out"]
    return out
